# revision 1
# baseline (speedup 1.0000x reference)
"""CrossAttentionFusion TRN2 kernel v2: fused single-pass, 8-core data parallel.

All-bf16 datapath (fp32 PSUM accumulation), no DRAM intermediates.
Per core (B_loc = 2048), per slab of 512 samples:
  - PE transposes feat1/feat2 (bf16, host-cast) -> x1t/x2t [f, fc, b]
  - Q^T,K^T form A (stat=W chunk, mov=x^T) -> qt/kt bf16 [d, h, 512]
  - V form B (stat=x2t chunk, mov=Wv) -> v_sb bf16 -> v_pack DMA -> vp [8s+g, grp, d]
  - middle per 16-sample group: 4 packed scores MMs (block-diag valid) ->
    exp (ACT) -> block-diag mask (DVE) -> ones-vector denom MM ->
    reciprocal (DVE) -> ctx MM -> scaled copy (ACT, per-partition scale)
  - ctx^T via PE transposes -> pass C out = ctx @ Wo -> DRAM (fp32)
"""

import sys

sys.path.insert(0, "/opt/trn_rl_repo")

import numpy as np
import concourse.bacc as bacc
import concourse.mybir as mybir
import concourse.tile as tile
from concourse.masks import make_identity

B = 16384
DIM = 1024
H = 8
HD = 128
SCALE = float(np.sqrt(HD))
NCORES = 8
B_LOC = B // NCORES  # 2048
SLAB = 512
NSLAB = B_LOC // SLAB  # 4
NSUB = SLAB // 128  # 4

F32 = mybir.dt.float32
BF16 = mybir.dt.bfloat16

_nc_cache = {}
TIME_LOOP_N = None
SKIP_MIDDLE = False   # debug: skip scores/softmax/ctx MMs (ctx garbage)
SKIP_TAIL = False     # debug: skip ctxT + pass C (out written from ctx garbage)


def build_nc():
    nc = bacc.Bacc(None)
    feat1t = nc.declare_dram_parameter("feat1t", [DIM, B_LOC], BF16, isOutput=False)
    feat2t = nc.declare_dram_parameter("feat2t", [DIM, B_LOC], BF16, isOutput=False)
    Wq = nc.declare_dram_parameter("Wq", [DIM, DIM], BF16, isOutput=False)
    Wk = nc.declare_dram_parameter("Wk", [DIM, DIM], BF16, isOutput=False)
    Wv = nc.declare_dram_parameter("Wv", [DIM, DIM], BF16, isOutput=False)
    Wo = nc.declare_dram_parameter("Wo", [DIM, DIM], BF16, isOutput=False)
    bdmask_d = nc.declare_dram_parameter("bdmaskd", [128, 128], BF16, isOutput=False)
    out = nc.declare_dram_parameter("out", [B_LOC, DIM], F32, isOutput=True)
    vdram = nc.dram_tensor("vdram", [B_LOC, DIM], BF16)

    with tile.TileContext(nc) as tc:
        with (
            tc.tile_pool(name="const", bufs=1) as cstpool,
            tc.tile_pool(name="w", bufs=1) as wpool,
            tc.tile_pool(name="feat", bufs=3) as fpool,
            tc.tile_pool(name="xt", bufs=2) as xtpool,
            tc.tile_pool(name="qk", bufs=2) as qkpool,
            tc.tile_pool(name="vsb", bufs=2) as vpool,
            tc.tile_pool(name="vp", bufs=9) as vppool,
            tc.tile_pool(name="e", bufs=6) as epool,
            tc.tile_pool(name="r", bufs=8) as rpool,
            tc.tile_pool(name="ctx", bufs=4) as ctxpool,
            tc.tile_pool(name="ctxT", bufs=2) as ctpool,
            tc.tile_pool(name="osb", bufs=4) as opool,
            tc.tile_pool(name="ps_work", bufs=3, space="PSUM") as pwork,
            tc.tile_pool(name="ps_sc", bufs=3, space="PSUM") as pscp,
            tc.tile_pool(name="ps_ctx", bufs=2, space="PSUM") as pctxp,
        ):
            ident_bf = cstpool.tile([128, 128], BF16, tag="identbf")
            make_identity(nc, ident_bf)
            bdmask4 = cstpool.tile([128, 4, 128], BF16, tag="bdmask4")
            for q in range(4):
                nc.gpsimd.dma_start(out=bdmask4[:, q, :], in_=bdmask_d[:, :])

            # weights as per-fc chunk tiles so matmuls can start after the
            # first chunk lands; fc-major issue order across two queues
            wts = {}
            for nmname, W in (("wq", Wq), ("wk", Wk), ("wv", Wv), ("wo", Wo)):
                wts[nmname] = [
                    wpool.tile([128, DIM], BF16, tag=f"{nmname}{fc}",
                               name=f"{nmname}{fc}")
                    for fc in range(8)
                ]
            for fc in range(8):
                for qi, (nmname, W) in enumerate(
                    (("wv", Wv), ("wq", Wq), ("wk", Wk), ("wo", Wo))
                ):
                    eng = nc.scalar if qi % 2 == 0 else nc.gpsimd
                    eng.dma_start(
                        out=wts[nmname][fc][:],
                        in_=W[fc * 128:(fc + 1) * 128, :],
                    )
            wq, wk, wv, wo = wts["wq"], wts["wk"], wts["wv"], wts["wo"]

            f1t_v = feat1t.rearrange("(c p) b -> p c b", p=128)
            f2t_v = feat2t.rearrange("(c p) b -> p c b", p=128)

            def emit_front(sl):
                base = sl * SLAB
                # ---- x^T tiles straight from host-transposed DRAM ----
                x1t = xtpool.tile([128, 8, SLAB], BF16, tag="x1t")
                x2t = xtpool.tile([128, 8, SLAB], BF16, tag="x2t")
                for c0 in range(0, 8, 2):
                    nc.sync.dma_start(
                        out=x2t[:, c0:c0 + 2, :],
                        in_=f2t_v[:, c0:c0 + 2, base:base + SLAB])
                    nc.sync.dma_start(
                        out=x1t[:, c0:c0 + 2, :],
                        in_=f1t_v[:, c0:c0 + 2, base:base + SLAB])
                vps = []
                for bt in range(NSUB):
                    b0 = base + bt * 128
                    # V for this sub (only needs its own x2t columns)
                    vsb = vpool.tile([128, DIM], BF16, tag="vsb")
                    for half in range(2):
                        pv = pwork.tile([128, 512], F32, tag="work")
                        for fc in range(8):
                            nc.tensor.matmul(
                                pv[:],
                                x2t[:, fc, bt * 128:(bt + 1) * 128],
                                wv[fc][:, half * 512:(half + 1) * 512],
                                start=(fc == 0),
                                stop=(fc == 7),
                            )
                        nc.vector.tensor_copy(
                            vsb[:, half * 512:(half + 1) * 512], pv[:]
                        )
                    # bounce V via DRAM; the scatter is a clean strided AP there:
                    # vp[p=8s+g, grp, d] = vdram[b0 + 16*grp + s, 128g + d]
                    lb0 = base + bt * 128
                    nc.gpsimd.dma_start(out=vdram[lb0:lb0 + 128, :], in_=vsb[:])
                    vp = vppool.tile([128, 8, 136], BF16, tag="vp", name="vp")
                    nc.vector.memset(vp[:, :, 128:129], 1.0)
                    nc.gpsimd.dma_start(
                        out=vp[:, :, 0:128],
                        in_=vdram[lb0:lb0 + 128, :].rearrange(
                            "(grp s) (g d) -> (s g) grp d", s=16, d=128),
                    )
                    vps.append(vp)

                # ---- Q^T / K^T (form A) ----
                qt = qkpool.tile([128, SLAB, 8], BF16, tag="qt")
                kt = qkpool.tile([128, SLAB, 8], BF16, tag="kt")
                def act_copy(dst_ap, src_ap):
                    nc.scalar.activation(
                        dst_ap, src_ap, mybir.ActivationFunctionType.Copy,
                        bias=0.0, scale=1.0,
                    )

                for (wmat, xt, dst, ceng) in (
                    (wq, x1t, qt, act_copy),
                    (wk, x2t, kt, nc.vector.tensor_copy),
                ):
                    for oc in range(8):
                        pq = pwork.tile([128, 512], F32, tag="work")
                        for fc in range(8):
                            nc.tensor.matmul(
                                pq[:],
                                wmat[fc][:, oc * 128:(oc + 1) * 128],
                                xt[:, fc, :],
                                start=(fc == 0),
                                stop=(fc == 7),
                            )
                        ceng(dst[:, :, oc], pq[:])
                return (sl, vps, qt, kt)

            def emit_mt(state):
                sl, vps, qt, kt = state
                base = sl * SLAB
                # ---- per sub: middle, ctx^T, out ----
                for bt in range(NSUB):
                    b0 = base + bt * 128
                    vp = vps[bt]
                    ctx_sb = ctxpool.tile([128, 8, 128], BF16, tag="ctx")
                    if SKIP_MIDDLE:
                        nc.vector.memset(ctx_sb[:, :, 0:1], 1.0)
                    for batch in range(0 if SKIP_MIDDLE else 2):
                        psc = pscp.tile([128, 512], F32, tag="psc")
                        for q in range(4):
                            g8 = 4 * batch + q
                            lb = bt * 128 + g8 * 16
                            qt_ap = qt[:, lb:lb + 16, :].rearrange("p i h -> p (i h)")
                            kt_ap = kt[:, lb:lb + 16, :].rearrange("p j g -> p (j g)")
                            nc.tensor.matmul(
                                psc[:, q * 128:(q + 1) * 128],
                                kt_ap,
                                qt_ap,
                                start=True, stop=True,
                            )
                        e = epool.tile([128, 4, 128], BF16, tag="e")
                        nc.scalar.activation(
                            e[:], psc[:].rearrange("p (q c) -> p q c", q=4),
                            mybir.ActivationFunctionType.Exp,
                            bias=0.0, scale=float(1.0 / SCALE),
                        )
                        em = epool.tile([128, 4, 128], BF16, tag="em")
                        nc.vector.tensor_mul(em[:], e[:], bdmask4[:])
                        for q in range(4):
                            g8 = 4 * batch + q
                            pc2 = pctxp.tile([128, 132], F32, tag="pc2")
                            nc.tensor.matmul(pc2[:, 0:129], em[:, q, :],
                                             vp[:, g8, 0:129],
                                             start=True, stop=True)
                            r = rpool.tile([128, 1], F32, tag="r")
                            nc.vector.reciprocal(r[:], pc2[:, 128:129])
                            nc.vector.tensor_scalar_mul(
                                ctx_sb[:, g8, :], pc2[:, 0:128], r[:, 0:1],
                            )

                    # ---- ctx^T ----
                    if SKIP_TAIL:
                        po = pwork.tile([128, 512], F32, tag="work")
                        nc.tensor.matmul(po[:], ctx_sb[:, 0, :],
                                         wo[0][:, 0:512], start=True, stop=True)
                        osb = opool.tile([128, 512], F32, tag="osb")
                        nc.scalar.activation(
                            osb[:], po[:], mybir.ActivationFunctionType.Copy,
                            bias=0.0, scale=1.0)
                        for half in range(2):
                            nc.scalar.dma_start(
                                out=out[b0:b0 + 128, half * 512:(half + 1) * 512],
                                in_=osb[:])
                        continue
                    ctxT = ctpool.tile([128, 8, 128], BF16, tag="ctxT")
                    for half in range(2):
                        ptb = pwork.tile([128, 512], BF16, tag="work")
                        for j in range(4):
                            nc.tensor.transpose(
                                ptb[:, j * 128:(j + 1) * 128],
                                ctx_sb[:, 4 * half + j, :],
                                ident_bf[:],
                            )
                        nc.vector.tensor_copy(
                            ctxT[:, 4 * half:4 * half + 4, :],
                            ptb[:].rearrange("p (j b) -> p j b", j=4),
                        )

                    # ---- pass C: out = ctx @ Wo ----
                    ctxT_v = ctxT[:].rearrange("d grp (s h) -> d (grp s) h", h=8)
                    for half in range(2):
                        po = pwork.tile([128, 512], F32, tag="work")
                        for h in range(8):
                            nc.tensor.matmul(
                                po[:],
                                ctxT_v[:, :, h],
                                wo[h][:, half * 512:(half + 1) * 512],
                                start=(h == 0),
                                stop=(h == 7),
                            )
                        osb = opool.tile([128, 512], F32, tag="osb")
                        nc.vector.tensor_copy(osb[:], po[:])
                        nc.scalar.dma_start(
                            out=out[b0:b0 + 128, half * 512:(half + 1) * 512],
                            in_=osb[:],
                        )

            def emit_all():
                prev = None
                for sl in range(NSLAB):
                    cur = emit_front(sl)
                    if prev is not None:
                        emit_mt(prev)
                    prev = cur
                emit_mt(prev)

            if TIME_LOOP_N:
                with tc.For_i(0, TIME_LOOP_N, 1) as _iv:
                    emit_all()
            else:
                emit_all()
    nc.compile()
    return nc


def _numpy_fallback(feat1, feat2, Wq, bq, Wk, bk, Wv, bv, Wo, bo):
    def sm(x):
        x = x - x.max(-1, keepdims=True)
        e = np.exp(x)
        return e / e.sum(-1, keepdims=True)

    b = feat1.shape[0]
    Q = (feat1 @ Wq + bq).reshape(b, H, HD)
    K = (feat2 @ Wk + bk).reshape(b, H, HD)
    V = (feat2 @ Wv + bv).reshape(b, H, HD)
    s = np.einsum("bhd,bgd->bhg", Q, K) / SCALE
    a = sm(s)
    ctx = np.einsum("bhg,bgd->bhd", a, V).reshape(b, DIM)
    return (ctx @ Wo + bo).astype(np.float32)


def make_in_maps(rng):
    import ml_dtypes
    s = 1.0 / np.sqrt(DIM)
    bf = lambda a: np.ascontiguousarray(a.astype(ml_dtypes.bfloat16))
    f1 = rng.standard_normal((B, DIM), dtype=np.float32)
    f2 = rng.standard_normal((B, DIM), dtype=np.float32)
    Ws = {n: bf(rng.standard_normal((DIM, DIM), dtype=np.float32) * s)
          for n in ("Wq", "Wk", "Wv", "Wo")}
    f1b, f2b = bf(f1), bf(f2)
    maps = []
    for c in range(NCORES):
        sl = slice(c * B_LOC, (c + 1) * B_LOC)
        maps.append({"feat1t": np.ascontiguousarray(f1b[sl].T),
                     "feat2t": np.ascontiguousarray(f2b[sl].T), **Ws,
                     "bdmaskd": _bdmask_np()})
    return maps


def _bdmask_np():
    import ml_dtypes
    m = np.zeros((128, 128), dtype=np.float32)
    for s in range(16):
        m[8 * s:8 * s + 8, 8 * s:8 * s + 8] = 1.0
    return np.ascontiguousarray(m.astype(ml_dtypes.bfloat16))


def kernel(feat1, feat2, Wq, bq, Wk, bk, Wv, bv, Wo, bo):
    import ml_dtypes

    feat1 = np.ascontiguousarray(np.asarray(feat1, dtype=np.float32))
    feat2 = np.ascontiguousarray(np.asarray(feat2, dtype=np.float32))
    Wq = np.ascontiguousarray(np.asarray(Wq, dtype=np.float32))
    Wk = np.ascontiguousarray(np.asarray(Wk, dtype=np.float32))
    Wv = np.ascontiguousarray(np.asarray(Wv, dtype=np.float32))
    Wo = np.ascontiguousarray(np.asarray(Wo, dtype=np.float32))
    bq, bk, bv, bo = (np.asarray(x, dtype=np.float32) for x in (bq, bk, bv, bo))
    if any(np.abs(x).max() > 0 for x in (bq, bk, bv, bo) if x.size):
        return _numpy_fallback(feat1, feat2, Wq, bq, Wk, bk, Wv, bv, Wo, bo)

    from concourse.bass_utils import run_bass_kernel_spmd

    if "nc" not in _nc_cache:
        _nc_cache["nc"] = build_nc()
    nc = _nc_cache["nc"]

    bfc = lambda a: np.ascontiguousarray(a.astype(ml_dtypes.bfloat16))
    f1b, f2b = bfc(feat1), bfc(feat2)
    Wqb, Wkb, Wvb, Wob = bfc(Wq), bfc(Wk), bfc(Wv), bfc(Wo)
    in_maps = []
    for c in range(NCORES):
        sl = slice(c * B_LOC, (c + 1) * B_LOC)
        in_maps.append({
            "feat1t": np.ascontiguousarray(f1b[sl].T),
            "feat2t": np.ascontiguousarray(f2b[sl].T),
            "Wq": Wqb, "Wk": Wkb, "Wv": Wvb, "Wo": Wob,
            "bdmaskd": _bdmask_np(),
        })
    res = run_bass_kernel_spmd(nc, in_maps, list(range(NCORES)))
    return np.concatenate([res.results[c]["out"] for c in range(NCORES)], axis=0)



# revision 27
# speedup vs baseline: 1.4667x; 1.4667x over previous
"""CrossAttentionFusion TRN2 kernel v3: fused single-pass, 8-core data parallel.

All-bf16 datapath (fp32 PSUM accumulation), no DRAM intermediates except the
V partition-scatter bounce. Per core (B_loc = 2048), per slab of 512 samples:
  - x1t/x2t [d, fc, b] bf16 straight from host-transposed DRAM
  - Q^T/K^T GEMMs (stat=W chunk, mov=x^T) -> qt/kt bf16 [d, head, b]
    (contiguous per-head layout -> unit-stride PSUM evacuations)
  - V GEMM (stat=x2t chunk, mov=Wv) -> vsb -> DRAM bounce -> vp [(g s), grp, d]
  - middle per 64-sample batch: rank-17 mask MM writes -336 off block-diag
    into the scores psum, 4 packed scores MMs accumulate, single ACT exp
    (masked entries underflow to ~0), per 3-group psum bank: ctx MMs with
    fused ones-column denominator, DVE reciprocal, ACT scaled-copy normalize
  - ctx^T via DMA xbar transposes (off the PE) -> out = ctx @ Wo -> DRAM (fp32)
"""

import sys

sys.path.insert(0, "/opt/trn_rl_repo")

import numpy as np
import concourse.bacc as bacc
import concourse.mybir as mybir
import concourse.tile as tile

B = 16384
DIM = 1024
H = 8
HD = 128
SCALE = float(np.sqrt(HD))
NCORES = 8
B_LOC = B // NCORES  # 2048
SLAB = 512
NSLAB = B_LOC // SLAB  # 4
NSUB = SLAB // 128  # 4
MASKP = 17  # rank of the block-diag mask matmul (1 + 16 sample-identity rows)
MBIG = 336.0  # pre-scale additive mask; exp(-336/11.31) ~ 1e-13

F32 = mybir.dt.float32
BF16 = mybir.dt.bfloat16

_nc_cache = {}
TIME_LOOP_N = None


def build_nc():
    nc = bacc.Bacc(None)
    feat1t = nc.declare_dram_parameter("feat1t", [DIM, B_LOC], BF16, isOutput=False)
    feat2t = nc.declare_dram_parameter("feat2t", [DIM, B_LOC], BF16, isOutput=False)
    Wq = nc.declare_dram_parameter("Wq", [DIM, DIM], BF16, isOutput=False)
    Wk = nc.declare_dram_parameter("Wk", [DIM, DIM], BF16, isOutput=False)
    Wv = nc.declare_dram_parameter("Wv", [DIM, DIM], BF16, isOutput=False)
    Wo = nc.declare_dram_parameter("Wo", [DIM, DIM], BF16, isOutput=False)
    maskab = nc.declare_dram_parameter("maskab", [MASKP, 640], BF16, isOutput=False)
    out = nc.declare_dram_parameter("out", [B_LOC, DIM], F32, isOutput=True)
    vdram = nc.dram_tensor("vdram", [B_LOC, DIM], BF16)

    with tile.TileContext(nc) as tc:
        with (
            tc.tile_pool(name="const", bufs=1) as cstpool,
            tc.tile_pool(name="w", bufs=1) as wpool,
            tc.tile_pool(name="xt", bufs=2) as xtpool,
            tc.tile_pool(name="qk", bufs=2) as qkpool,
            tc.tile_pool(name="vsb", bufs=2) as vpool,
            tc.tile_pool(name="vp", bufs=9) as vppool,
            tc.tile_pool(name="e", bufs=4) as epool,
            tc.tile_pool(name="r", bufs=4) as rpool,
            tc.tile_pool(name="rb", bufs=4) as rbpool,
            tc.tile_pool(name="ctxT", bufs=3) as ctpool,
            tc.tile_pool(name="osb", bufs=4) as opool,
            tc.tile_pool(name="ps_work", bufs=2, space="PSUM") as pwork,
            tc.tile_pool(name="ps_sc", bufs=2, space="PSUM") as pscp,
            tc.tile_pool(name="ps_ctx", bufs=2, space="PSUM") as pctxp,
            tc.tile_pool(name="ps_out", bufs=2, space="PSUM") as poutp,
        ):
            mk = cstpool.tile([MASKP, 640], BF16, tag="maskab")
            nc.scalar.dma_start(out=mk[:], in_=maskab[:, :])
            ones_col = cstpool.tile([128, 1], BF16, tag="ones")
            nc.vector.memset(ones_col[:], 1.0)

            # weights as per-fc chunk tiles so matmuls can start after the
            # first chunk lands; fc-major issue order across two queues
            wts = {}
            for nmname, W in (("wq", Wq), ("wk", Wk), ("wv", Wv), ("wo", Wo)):
                wts[nmname] = [
                    wpool.tile([128, DIM], BF16, tag=f"{nmname}{fc}",
                               name=f"{nmname}{fc}")
                    for fc in range(8)
                ]
            for fc in range(8):
                for qi, (nmname, W) in enumerate(
                    (("wv", Wv), ("wq", Wq), ("wk", Wk), ("wo", Wo))
                ):
                    eng = nc.scalar if qi % 2 == 0 else nc.gpsimd
                    eng.dma_start(
                        out=wts[nmname][fc][:],
                        in_=W[fc * 128:(fc + 1) * 128, :],
                    )
            wq, wk, wv, wo = wts["wq"], wts["wk"], wts["wv"], wts["wo"]

            f1t_v = feat1t.rearrange("(c p) b -> p c b", p=128)
            f2t_v = feat2t.rearrange("(c p) b -> p c b", p=128)

            def emit_front(sl):
                base = sl * SLAB
                # ---- x^T tiles straight from host-transposed DRAM ----
                x1t = xtpool.tile([128, 8, SLAB], BF16, tag="x1t")
                x2t = xtpool.tile([128, 8, SLAB], BF16, tag="x2t")
                for c0 in range(0, 8, 2):
                    nc.sync.dma_start(
                        out=x2t[:, c0:c0 + 2, :],
                        in_=f2t_v[:, c0:c0 + 2, base:base + SLAB])
                    nc.scalar.dma_start(
                        out=x1t[:, c0:c0 + 2, :],
                        in_=f1t_v[:, c0:c0 + 2, base:base + SLAB])
                vps = []
                for bt in range(NSUB):
                    # V for this sub (only needs its own x2t columns)
                    vsb = vpool.tile([128, DIM], BF16, tag="vsb")
                    for half in range(2):
                        pv = pwork.tile([128, 512], F32, tag="work")
                        for fc in range(8):
                            nc.tensor.matmul(
                                pv[:],
                                x2t[:, fc, bt * 128:(bt + 1) * 128],
                                wv[fc][:, half * 512:(half + 1) * 512],
                                start=(fc == 0),
                                stop=(fc == 7),
                            )
                        nc.vector.tensor_copy(
                            vsb[:, half * 512:(half + 1) * 512], pv[:]
                        )
                    # bounce V via DRAM; the scatter is a clean strided AP:
                    # vp[p=16g+s, grp, d] = vdram[b0 + 16*grp + s, 128g + d]
                    lb0 = base + bt * 128
                    nc.gpsimd.dma_start(out=vdram[lb0:lb0 + 128, :], in_=vsb[:])
                    vp = vppool.tile([128, 8, 128], BF16, tag="vp", name="vp")
                    # vp[p=16g+s, grp, d] = V[b0 + 16*grp + s, 128g + d]
                    for g in range(8):
                        eng = nc.gpsimd if g % 2 == 0 else nc.sync
                        eng.dma_start(
                            out=vp[g * 16:(g + 1) * 16, :, :],
                            in_=vdram[lb0:lb0 + 128,
                                      g * 128:(g + 1) * 128].rearrange(
                                "(grp s) d -> s grp d", s=16),
                        )
                    vps.append(vp)

                # ---- Q^T: qt [d, head, b]; K^T: kt [d, G, g, j] packed ----
                # kt[d, G, g, j] = K[sample G*16+j, head-dim 128g+d]: the
                # (g j)-contiguous pack makes both the evacuation (dst runs
                # of 16, step 1) and the scores stationary slice (single
                # merged free dim) fast.
                qt = qkpool.tile([128, 8, SLAB], BF16, tag="qt")
                kt = qkpool.tile([128, SLAB // 16, 8, 16], BF16, tag="kt")
                def act_copy(dst_ap, src_ap):
                    nc.scalar.activation(
                        dst_ap, src_ap, mybir.ActivationFunctionType.Copy,
                        bias=0.0, scale=1.0,
                    )

                for oc in range(8):
                    pq = pwork.tile([128, 512], F32, tag="work")
                    for fc in range(8):
                        nc.tensor.matmul(
                            pq[:],
                            wq[fc][:, oc * 128:(oc + 1) * 128],
                            x1t[:, fc, :],
                            start=(fc == 0),
                            stop=(fc == 7),
                        )
                    act_copy(qt[:, oc, :], pq[:])
                for oc in range(8):
                    pq = pwork.tile([128, 512], F32, tag="work")
                    for fc in range(8):
                        nc.tensor.matmul(
                            pq[:],
                            wk[fc][:, oc * 128:(oc + 1) * 128],
                            x2t[:, fc, :],
                            start=(fc == 0),
                            stop=(fc == 7),
                        )
                    nc.vector.tensor_copy(
                        kt[:, :, oc, :],
                        pq[:].rearrange("p (G j) -> p G j", j=16),
                    )
                return (sl, vps, qt, kt)

            def emit_mt(state):
                sl, vps, qt, kt = state
                base = sl * SLAB
                for bt in range(NSUB):
                    b0 = base + bt * 128
                    vp = vps[bt]
                    # ctxT[d, g8, (i h)] = ctx[sample g8*16+i, 128h+d]
                    ctxT = ctpool.tile([128, 8, 128], BF16, tag="ctxT")
                    for batch in range(2):
                        psc = pscp.tile([128, 512], F32, tag="psc")
                        # block-diag mask: psc = -MBIG everywhere except the
                        # 16x16-tiled sample-diagonal (rank-17 matmul)
                        nc.tensor.matmul(
                            psc[:], mk[:, 0:128], mk[:, 128:640],
                            start=True, stop=False,
                        )
                        for q in range(4):
                            g8 = 4 * batch + q
                            lb = bt * 128 + g8 * 16
                            kt_ap = kt[:, bt * 8 + g8, :, :].rearrange(
                                "p g j -> p (g j)")
                            # moving columns packed (h i) = h*16+i
                            qt_ap = qt[:, :, lb:lb + 16]
                            nc.tensor.matmul(
                                psc[:, q * 128:(q + 1) * 128],
                                kt_ap,
                                qt_ap,
                                start=False, stop=(q == 3),
                            )
                        e = epool.tile([128, 4, 128], BF16, tag="e")
                        nc.scalar.activation(
                            e[:], psc[:].rearrange("p (q c) -> p q c", q=4),
                            mybir.ActivationFunctionType.Exp,
                            bias=0.0, scale=float(1.0 / SCALE),
                        )
                        # denominators for the whole batch: column sums of e,
                        # parked in the (dead after exp) psc bank, partition 0
                        nc.tensor.matmul(
                            psc[0:1, :], ones_col[:],
                            e[:].rearrange("p q c -> p (q c)"),
                            start=True, stop=True, skip_group_check=True,
                        )
                        rt = rpool.tile([1, 512], F32, tag="rt")
                        nc.vector.reciprocal_approx_fast(rt[:], psc[0:1, :])
                        rb = rbpool.tile([128, 512], F32, tag="rb")
                        nc.gpsimd.partition_broadcast(rb[:], rt[:])
                        # ctx^T direct: stat = vp (ready early), moving = raw
                        # e; normalization happens during evacuation below
                        pctxT = pctxp.tile([128, 4, 128], F32, tag="pctxT")
                        for q in range(4):
                            g8 = 4 * batch + q
                            nc.tensor.matmul(
                                pctxT[:, q, :], vp[:, g8, :], e[:, q, :],
                                start=True, stop=True,
                            )
                        # scatter into per-head layout ctxT[d, h, sample]
                        # (sample = (4*batch+q)*16 + i) so the out-GEMM
                        # stationary slices are contiguous
                        nc.vector.tensor_mul(
                            ctxT[:, :, 64 * batch:64 * batch + 64].rearrange(
                                "p h (q i) -> p h q i", q=4),
                            pctxT[:].rearrange("p q (h i) -> p h q i", h=8),
                            rb[:].rearrange("p (q h i) -> p h q i", q=4, h=8),
                        )

                    # ---- out = ctx @ Wo ----
                    for half in range(2):
                        po = poutp.tile([128, 512], F32, tag="pout")
                        for h in range(8):
                            ct_ap = ctxT[:, h, :]
                            nc.tensor.matmul(
                                po[:],
                                ct_ap,
                                wo[h][:, half * 512:(half + 1) * 512],
                                start=(h == 0),
                                stop=(h == 7),
                            )
                        osb = opool.tile([128, 512], F32, tag="osb")
                        nc.scalar.activation(
                            osb[:], po[:], mybir.ActivationFunctionType.Copy,
                            bias=0.0, scale=1.0,
                        )
                        nc.sync.dma_start(
                            out=out[b0:b0 + 128, half * 512:(half + 1) * 512],
                            in_=osb[:],
                        )

            def emit_all():
                prev = None
                for sl in range(NSLAB):
                    cur = emit_front(sl)
                    if prev is not None:
                        emit_mt(prev)
                    prev = cur
                emit_mt(prev)

            if TIME_LOOP_N:
                with tc.For_i(0, TIME_LOOP_N, 1) as _iv:
                    emit_all()
            else:
                emit_all()
    nc.compile()
    return nc


def _numpy_fallback(feat1, feat2, Wq, bq, Wk, bk, Wv, bv, Wo, bo):
    def sm(x):
        x = x - x.max(-1, keepdims=True)
        e = np.exp(x)
        return e / e.sum(-1, keepdims=True)

    b = feat1.shape[0]
    Q = (feat1 @ Wq + bq).reshape(b, H, HD)
    K = (feat2 @ Wk + bk).reshape(b, H, HD)
    V = (feat2 @ Wv + bv).reshape(b, H, HD)
    s = np.einsum("bhd,bgd->bhg", Q, K) / SCALE
    a = sm(s)
    ctx = np.einsum("bhg,bgd->bhd", a, V).reshape(b, DIM)
    return (ctx @ Wo + bo).astype(np.float32)


def _maskab_np():
    import ml_dtypes
    m = np.zeros((MASKP, 640), dtype=np.float32)
    m[0, 0:128] = 1.0
    m[0, 128:640] = -MBIG
    for s in range(16):
        for g in range(8):
            m[1 + s, g * 16 + s] = 1.0  # A: [j == s] at col (g j)
        for blk in range(4):
            for h in range(8):
                m[1 + s, 128 + blk * 128 + h * 16 + s] = MBIG  # B: [i == s]
    return np.ascontiguousarray(m.astype(ml_dtypes.bfloat16))


def make_in_maps(rng):
    import ml_dtypes
    s = 1.0 / np.sqrt(DIM)
    bf = lambda a: np.ascontiguousarray(a.astype(ml_dtypes.bfloat16))
    f1 = rng.standard_normal((B, DIM), dtype=np.float32)
    f2 = rng.standard_normal((B, DIM), dtype=np.float32)
    Ws = {n: bf(rng.standard_normal((DIM, DIM), dtype=np.float32) * s)
          for n in ("Wq", "Wk", "Wv", "Wo")}
    f1b, f2b = bf(f1), bf(f2)
    maps = []
    for c in range(NCORES):
        sl = slice(c * B_LOC, (c + 1) * B_LOC)
        maps.append({"feat1t": np.ascontiguousarray(f1b[sl].T),
                     "feat2t": np.ascontiguousarray(f2b[sl].T), **Ws,
                     "maskab": _maskab_np()})
    return maps


def kernel(feat1, feat2, Wq, bq, Wk, bk, Wv, bv, Wo, bo):
    import ml_dtypes

    feat1 = np.ascontiguousarray(np.asarray(feat1, dtype=np.float32))
    feat2 = np.ascontiguousarray(np.asarray(feat2, dtype=np.float32))
    Wq = np.ascontiguousarray(np.asarray(Wq, dtype=np.float32))
    Wk = np.ascontiguousarray(np.asarray(Wk, dtype=np.float32))
    Wv = np.ascontiguousarray(np.asarray(Wv, dtype=np.float32))
    Wo = np.ascontiguousarray(np.asarray(Wo, dtype=np.float32))
    bq, bk, bv, bo = (np.asarray(x, dtype=np.float32) for x in (bq, bk, bv, bo))
    if any(np.abs(x).max() > 0 for x in (bq, bk, bv, bo) if x.size):
        return _numpy_fallback(feat1, feat2, Wq, bq, Wk, bk, Wv, bv, Wo, bo)

    from concourse.bass_utils import run_bass_kernel_spmd

    if "nc" not in _nc_cache:
        _nc_cache["nc"] = build_nc()
    nc = _nc_cache["nc"]

    bfc = lambda a: np.ascontiguousarray(a.astype(ml_dtypes.bfloat16))
    f1b, f2b = bfc(feat1), bfc(feat2)
    Wqb, Wkb, Wvb, Wob = bfc(Wq), bfc(Wk), bfc(Wv), bfc(Wo)
    in_maps = []
    for c in range(NCORES):
        sl = slice(c * B_LOC, (c + 1) * B_LOC)
        in_maps.append({
            "feat1t": np.ascontiguousarray(f1b[sl].T),
            "feat2t": np.ascontiguousarray(f2b[sl].T),
            "Wq": Wqb, "Wk": Wkb, "Wv": Wvb, "Wo": Wob,
            "maskab": _maskab_np(),
        })
    res = run_bass_kernel_spmd(nc, in_maps, list(range(NCORES)))
    return np.concatenate([res.results[c]["out"] for c in range(NCORES)], axis=0)
